# revision 9
# baseline (speedup 1.0000x reference)
"""Correct-and-Smooth label propagation on 8 Trainium2 NeuronCores.

V5 architecture (replaces the per-slot INDIRECT1D gather, which was
Pool-descriptor-generation bound at ~1 us/slot):
  * nodes dest-sharded across 8 cores; full fp16 state table replicated in
    SBUF, windows of 128 consecutive (relabeled) node rows.
  * per step, each core expands its in-edge messages with TensorE: one
    [128x128] fp8 0/1 selection matmul per block gathers up to 128 source
    rows of one window into PSUM (stationary load is cheap, no DMA
    descriptors per edge).
  * DVE multiplies PSUM blocks by per-edge norms and casts to fp16 messages.
  * dma_scatter_add permutes messages into a 2-slot-per-dest DRAM staging
    table (int16 indices; each call's (dest,slot) pairs are unique so the
    CCE add never sees in-call duplicate rows; calls accumulate).
  * window-overflow and slot-collision residue goes through a bounded
    INDIRECT1D straggler lane (SG groups of 128 rows).
  * staging is read back, slots summed, alpha-mix + clip/softmax applied in
    fp32, and the fp16 shard AllGathered into the next table.
"""
import numpy as np

N, E, C, M = 100000, 1600000, 40, 8
NPC, TPC = 12544, 98
NWIN = (M * NPC) // 128          # 784 source windows
NBW = 2                          # blocks per window (uniform schedule)
NB = NWIN * NBW                  # 1568 matmul blocks
CBLK = 32                        # blocks per scatter call
NCALLS = NB // CBLK              # 49
PSG = 8                          # blocks per PSUM group
SG = 64                          # straggler gather groups (128 rows each)
NSC = 4                          # straggler scatter calls (SG/NSC groups each)
SGC = SG // NSC
TRASH = NPC * 2                  # staging trash row
SROWS = NPC * 2 + 1              # addressed staging rows (25089)
SPAD = 25216                     # staging rows padded to 128 multiple
ALPHA_C, NUM_C = 0.9, 10
ALPHA_S, NUM_S = 0.8, 10

_cache = {}


def _preprocess(y_true, y_soft, spread_mask, edge_index, edge_weight):
    import ml_dtypes
    y_true = np.asarray(y_true)
    y_soft = np.asarray(y_soft, dtype=np.float32)
    spread_mask = np.asarray(spread_mask).astype(bool)
    row = np.asarray(edge_index[0], dtype=np.int64)
    col = np.asarray(edge_index[1], dtype=np.int64)
    w = np.asarray(edge_weight, dtype=np.float32)

    deg = np.bincount(row, weights=w.astype(np.float64), minlength=N).astype(np.float32)
    dis = np.where(deg > 0, 1.0 / np.sqrt(np.where(deg > 0, deg, 1.0)), 0.0).astype(np.float32)
    norm = (dis[row] * w * dis[col]).astype(np.float32)

    indeg = np.bincount(row, minlength=N)
    order = np.argsort(indeg, kind="stable")
    ranks = np.arange(N)
    core_of = np.empty(N, np.int64)
    pos_of = np.empty(N, np.int64)
    core_of[order] = ranks % M
    pos_of[order] = ranks // M          # < 12500 <= NPC
    new_id = core_of * NPC + pos_of     # local j = t*128+p handled below

    # local j layout: j = t*128 + p  (partition = j % 128)
    nrow = new_id[row]
    ncol = new_id[col]

    # state tensors (padded node space, new_id order)
    y_oh = np.zeros((N, C), np.float32)
    y_oh[np.arange(N), y_true] = 1.0
    maskf = spread_mask[:, None]
    err = np.where(maskf, y_oh - y_soft, 0.0).astype(np.float32)
    sigma = float(np.abs(err).sum(dtype=np.float64) / spread_mask.sum())

    x0 = np.zeros((M * NPC, C), np.float32)
    x0[new_id] = err
    A = np.zeros((M * NPC, C), np.float32)
    A[new_id] = np.where(maskf, y_oh, y_soft)
    B = np.zeros((M * NPC,), np.float32)
    B[new_id] = (~spread_mask).astype(np.float32)

    def shard_mat(full, k):  # [NPC(,C)] j-order -> [128, TPC*(C)]
        s = full[k * NPC:(k + 1) * NPC]
        if s.ndim == 1:
            return np.ascontiguousarray(s.reshape(TPC, 128).T)
        return np.ascontiguousarray(
            s.reshape(TPC, 128, C).transpose(1, 0, 2).reshape(128, TPC * C))

    x16 = x0.astype(np.float16)
    xtab0 = np.ascontiguousarray(
        x16.reshape(NWIN, 128, C).transpose(1, 0, 2).reshape(128, NWIN * C))

    fp8np = ml_dtypes.float8_e4m3
    per_core = []
    for k in range(M):
        em = np.where((nrow // NPC) == k)[0]
        dj = (nrow[em] - k * NPC).astype(np.int64)
        sr = ncol[em].astype(np.int64)
        nm = norm[em]
        win = sr // 128
        o = np.argsort(win, kind="stable")
        dj, sr, nm, win = dj[o], sr[o], nm[o], win[o]
        rho = (sr % 128).astype(np.int64)
        nmsg = len(dj)

        fill = np.zeros(NB, np.int32)
        cnt = np.zeros((NCALLS, NPC), np.int8)
        blk_a = np.empty(nmsg, np.int32)
        pos_a = np.empty(nmsg, np.int32)
        sig_a = np.empty(nmsg, np.int8)
        placed = np.zeros(nmsg, bool)
        strag = []
        cw = CBLK
        for i in range(nmsg):
            d = dj[i]
            b0 = 2 * win[i]
            ok = False
            for b in (b0, b0 + 1):
                cc = b // cw
                if fill[b] < 128 and cnt[cc, d] < 2:
                    blk_a[i] = b
                    pos_a[i] = fill[b]
                    sig_a[i] = cnt[cc, d]
                    fill[b] += 1
                    cnt[cc, d] += 1
                    placed[i] = True
                    ok = True
                    break
            if not ok:
                strag.append(i)

        # straggler lane: fill groups left to right; a group holds up to 128
        # rows; each scatter call (SGC groups) may see a dest at most twice.
        scnt = np.zeros((NSC, NPC), np.int8)
        gfill = np.zeros(SG, np.int32)
        s_lane = np.empty(len(strag), np.int32)
        s_grp = np.empty(len(strag), np.int32)
        s_sig = np.empty(len(strag), np.int8)
        if len(strag) > SG * 128:
            raise RuntimeError(f"straggler overflow: {len(strag)} > {SG*128}")
        cfill = np.zeros(NSC, np.int64)
        for si, i in enumerate(strag):
            d = dj[i]
            # among calls where this dest has headroom, pick the emptiest
            elig = [s for s in range(NSC)
                    if scnt[s, d] < 2 and cfill[s] < SGC * 128]
            if not elig:
                raise RuntimeError(
                    f"straggler placement failed core={k} "
                    f"n={len(strag)} si={si}")
            sc = min(elig, key=lambda s: cfill[s])
            g = sc * SGC + int(cfill[sc]) // 128
            s_grp[si], s_lane[si], s_sig[si] = g, int(cfill[sc]) % 128, scnt[sc, d]
            scnt[sc, d] += 1
            cfill[sc] += 1
            gfill[g] += 1

        # W, nrm, idx arrays
        W8 = np.zeros((128, NB * 128), fp8np)
        pm = placed
        W8[rho[pm], blk_a[pm] * 128 + pos_a[pm]] = fp8np(1.0)
        nrm32 = np.zeros((128, NB), np.float32)
        nrm32[pos_a[pm], blk_a[pm]] = nm[pm]

        idxv = np.full(NB * 128, TRASH, np.int64)
        q = blk_a[pm] * 128 + pos_a[pm]
        idxv[q] = dj[pm] * 2 + sig_a[pm]
        # per call wrapped-16: call c covers slots [c*4096, (c+1)*4096)
        iw = idxv.reshape(NCALLS, CBLK * 128 // 16, 16).transpose(0, 2, 1)
        idx16 = np.tile(iw.reshape(NCALLS, 16, 256).transpose(1, 0, 2)
                        .reshape(16, NCALLS * 256), (8, 1)).astype(np.int16)

        soff = np.zeros((128, SG), np.int32)
        snrm = np.zeros((128, SG), np.float32)
        sidxv = np.full(SG * 128, TRASH, np.int64)
        if len(strag):
            ii = np.array(strag)
            soff[s_lane, s_grp] = sr[ii].astype(np.int32)
            snrm[s_lane, s_grp] = nm[ii]
            sidxv[s_grp * 128 + s_lane] = dj[ii] * 2 + s_sig
        siw = sidxv.reshape(NSC, SGC * 128 // 16, 16).transpose(0, 2, 1)
        sidx16 = np.tile(siw.reshape(NSC, 16, SGC * 8).transpose(1, 0, 2)
                         .reshape(16, NSC * SGC * 8), (8, 1)).astype(np.int16)

        per_core.append(dict(
            w8=W8, nrm=nrm32, idx=idx16, soff=soff, snrm=snrm, sidx=sidx16,
            xo0=shard_mat(x0, k), amat=shard_mat(A, k), bvec=shard_mat(B, k),
        ))

    return dict(per_core=per_core, xtab0=xtab0, xrep0=x16, sigma=sigma,
                new_id=new_id)


def _build_program(sigma):
    import concourse.bass as bass
    import concourse.bacc as bacc
    import concourse.mybir as mybir
    import concourse.tile as tile

    f16 = mybir.dt.float16
    f32 = mybir.dt.float32
    fp8 = mybir.dt.float8e4
    i16 = mybir.dt.int16
    i32 = mybir.dt.int32
    Alu = mybir.AluOpType
    Act = mybir.ActivationFunctionType
    X = mybir.AxisListType.X

    nc = bacc.Bacc("TRN2", target_bir_lowering=False, debug=False,
                   enable_asserts=True, num_devices=M)
    w8_d = nc.dram_tensor("w8", [128, NB * 128], fp8, kind="ExternalInput")
    nrm_d = nc.dram_tensor("nrm", [128, NB], f32, kind="ExternalInput")
    idx_d = nc.dram_tensor("idx", [128, NCALLS * 256], i16, kind="ExternalInput")
    soff_d = nc.dram_tensor("soff", [128, SG], i32, kind="ExternalInput")
    snrm_d = nc.dram_tensor("snrm", [128, SG], f32, kind="ExternalInput")
    sidx_d = nc.dram_tensor("sidx", [128, NSC * SGC * 8], i16, kind="ExternalInput")
    xtab0_d = nc.dram_tensor("xtab0", [128, NWIN * C], f16, kind="ExternalInput")
    xrep0_d = nc.dram_tensor("xrep0", [M * NPC, C], f16, kind="ExternalInput")
    xo0_d = nc.dram_tensor("xo0", [128, TPC * C], f32, kind="ExternalInput")
    a_d = nc.dram_tensor("amat", [128, TPC * C], f32, kind="ExternalInput")
    b_d = nc.dram_tensor("bvec", [128, TPC], f32, kind="ExternalInput")
    out_d = nc.dram_tensor("out", [128, TPC * C], f32, kind="ExternalOutput")

    NSTEPS = NUM_C + NUM_S
    with tile.TileContext(nc) as tc:
        with (
            tc.tile_pool(name="stat", bufs=1) as stat,
            tc.tile_pool(name="tabp", bufs=1) as tabp,
            tc.tile_pool(name="wpool", bufs=2) as wpool,
            tc.tile_pool(name="ipool", bufs=2) as ipool,
            tc.tile_pool(name="mpool", bufs=2) as mpool,
            tc.tile_pool(name="psum", bufs=4, space="PSUM") as psum,
            tc.tile_pool(name="spool", bufs=2) as spool,
            tc.tile_pool(name="agp", bufs=1) as agp,
            tc.tile_pool(name="smp", bufs=2) as smp,
            tc.tile_pool(name="sgp", bufs=1) as sgp,
            tc.tile_pool(name="dpool", bufs=2, space="DRAM") as dpool,
        ):
            nrm_t = stat.tile([128, NB], f32)
            nc.sync.dma_start(out=nrm_t[:], in_=nrm_d[:])
            soff_t = stat.tile([128, SG], i32)
            nc.sync.dma_start(out=soff_t[:], in_=soff_d[:])
            snrm_t = stat.tile([128, SG], f32)
            nc.sync.dma_start(out=snrm_t[:], in_=snrm_d[:])
            sidx_t = stat.tile([128, NSC * SGC * 8], i16)
            nc.sync.dma_start(out=sidx_t[:], in_=sidx_d[:])
            a_t = stat.tile([128, TPC * C], f32)
            nc.sync.dma_start(out=a_t[:], in_=a_d[:])
            b_t = stat.tile([128, TPC], f32)
            nc.sync.dma_start(out=b_t[:], in_=b_d[:])
            zt = stat.tile([128, 3152], f16)
            nc.vector.memset(zt[:], 0.0)

            xcur = spool.tile([128, TPC * C], f32, tag="xst")
            nc.sync.dma_start(out=xcur[:], in_=xo0_d[:])
            xtab = tabp.tile([128, NWIN * C], f16, tag="xtab")
            nc.sync.dma_start(out=xtab[:], in_=xtab0_d[:])
            src = xrep0_d.ap()

            for step in range(NSTEPS):
                phase1 = step < NUM_C
                alpha = ALPHA_C if phase1 else ALPHA_S

                stag = dpool.tile([SPAD, 128], f16, tag="stag")
                sv = stag[:].rearrange("(p a) r -> p (a r)", p=128)
                for z in range(8):
                    nc.sync.dma_start(out=sv[:, z * 3152:(z + 1) * 3152],
                                      in_=zt[:])

                for call in range(NCALLS):
                    wch = wpool.tile([128, CBLK * 128], fp8, tag="w")
                    nc.sync.dma_start(
                        out=wch[:], in_=w8_d[:, call * CBLK * 128:(call + 1) * CBLK * 128])
                    ich = ipool.tile([128, 256], i16, tag="i")
                    nc.sync.dma_start(
                        out=ich[:], in_=idx_d[:, call * 256:(call + 1) * 256])
                    msg = mpool.tile([128, CBLK, C], f16, tag="m")
                    for grp in range(CBLK // PSG):
                        ps = psum.tile([128, PSG, C], f32, tag="ps")
                        for ib in range(PSG):
                            b = call * CBLK + grp * PSG + ib
                            tau = b // NBW
                            nc.tensor.matmul(
                                ps[:, ib],
                                wch[:, (grp * PSG + ib) * 128:(grp * PSG + ib + 1) * 128],
                                xtab[:, tau * C:(tau + 1) * C],
                                start=True, stop=True)
                        b0 = call * CBLK + grp * PSG
                        nv = nrm_t[:, b0:b0 + PSG].unsqueeze(-1).broadcast_to(
                            [128, PSG, C])
                        nc.vector.tensor_tensor(
                            out=msg[:, grp * PSG:(grp + 1) * PSG],
                            in0=ps[:], in1=nv, op=Alu.mult)
                    nc.gpsimd.dma_scatter_add(
                        out_ap=stag[:SROWS, :C],
                        in_ap=msg[:],
                        idxs_ap=ich[:],
                        num_idxs=CBLK * 128,
                        num_idxs_reg=CBLK * 128,
                        elem_size=C,
                        elem_step=128,
                    )

                # straggler lane: indirect row gathers from the DRAM table
                sgb = sgp.tile([128, SG, C], f16, tag="sg")
                for g in range(SG):
                    nc.gpsimd.indirect_dma_start(
                        out=sgb[:, g], out_offset=None, in_=src,
                        in_offset=bass.IndirectOffsetOnAxis(
                            ap=soff_t[:, g:g + 1], axis=0))
                sgm = sgp.tile([128, SG, C], f16, tag="sgm")
                snv = snrm_t[:].unsqueeze(-1).broadcast_to([128, SG, C])
                nc.vector.tensor_tensor(out=sgm[:], in0=sgb[:], in1=snv,
                                        op=Alu.mult)
                for sc in range(NSC):
                    nc.gpsimd.dma_scatter_add(
                        out_ap=stag[:SROWS, :C],
                        in_ap=sgm[:, sc * SGC:(sc + 1) * SGC],
                        idxs_ap=sidx_t[:, sc * SGC * 8:(sc + 1) * SGC * 8],
                        num_idxs=SGC * 128,
                        num_idxs_reg=SGC * 128,
                        elem_size=C,
                        elem_step=128,
                    )

                # read staging back and sum the two slots
                agt = agp.tile([128, TPC, 2, C], f16, tag="agt")
                rbv = stag[:NPC * 2].rearrange("(t p s) r -> p t s r", p=128, s=2)
                nc.sync.dma_start(out=agt[:, :, 0], in_=rbv[:, :, 0, :C])
                nc.sync.dma_start(out=agt[:, :, 1], in_=rbv[:, :, 1, :C])
                agg = agp.tile([128, TPC * C], f32, tag="agg")
                av = agg[:].rearrange("p (t c) -> p t c", c=C)
                nc.vector.tensor_tensor(out=av, in0=agt[:, :, 0], in1=agt[:, :, 1],
                                        op=Alu.add)

                xn = spool.tile([128, TPC * C], f32, tag="xst")
                nc.scalar.activation(xn[:], xcur[:], Act.Copy, scale=float(1 - alpha))
                nc.vector.tensor_scalar_mul(out=agg[:], in0=agg[:],
                                            scalar1=float(alpha))
                nc.vector.tensor_add(out=xn[:], in0=xn[:], in1=agg[:])

                if phase1:
                    nc.vector.tensor_scalar_max(out=xn[:], in0=xn[:], scalar1=-1.0)
                    nc.vector.tensor_scalar_min(out=xn[:], in0=xn[:], scalar1=1.0)
                else:
                    xv = xn[:].rearrange("p (t c) -> p t c", c=C)
                    rm = smp.tile([128, TPC], f32, tag="rm")
                    nc.vector.tensor_reduce(out=rm[:], in_=xv, axis=X, op=Alu.max)
                    rmb = rm[:].unsqueeze(-1).broadcast_to([128, TPC, C])
                    nc.vector.tensor_tensor(out=xv, in0=xv, in1=rmb, op=Alu.subtract)
                    nc.scalar.activation(xn[:], xn[:], Act.Exp)
                    ss = smp.tile([128, TPC], f32, tag="ss")
                    nc.vector.tensor_reduce(out=ss[:], in_=xv, axis=X, op=Alu.add)
                    nc.vector.reciprocal(out=ss[:], in_=ss[:])
                    ssb = ss[:].unsqueeze(-1).broadcast_to([128, TPC, C])
                    nc.vector.tensor_tensor(out=xv, in0=xv, in1=ssb, op=Alu.mult)

                if step == NUM_C - 1:
                    xv = xn[:].rearrange("p (t c) -> p t c", c=C)
                    den = smp.tile([128, TPC], f32, tag="den")
                    nc.vector.tensor_reduce(out=den[:], in_=xv, axis=X, op=Alu.add,
                                            apply_absolute_value=True)
                    raw = smp.tile([128, TPC], f32, tag="raw")
                    nc.vector.reciprocal(out=raw[:], in_=den[:])
                    nc.vector.tensor_scalar_mul(out=raw[:], in0=raw[:],
                                                scalar1=float(sigma))
                    nc.vector.tensor_scalar_min(out=raw[:], in0=raw[:],
                                                scalar1=1001.0)
                    m1 = smp.tile([128, TPC], f32, tag="m1")
                    nc.vector.tensor_scalar(out=m1[:], in0=den[:], scalar1=0.0,
                                            scalar2=None, op0=Alu.is_gt)
                    m2 = smp.tile([128, TPC], f32, tag="m2")
                    nc.vector.tensor_scalar(out=m2[:], in0=raw[:], scalar1=1000.0,
                                            scalar2=None, op0=Alu.is_le)
                    nc.vector.tensor_tensor(out=m1[:], in0=m1[:], in1=m2[:],
                                            op=Alu.mult)
                    scl = smp.tile([128, TPC], f32, tag="scl")
                    nc.vector.tensor_scalar_add(out=raw[:], in0=raw[:], scalar1=-1.0)
                    nc.vector.tensor_tensor(out=scl[:], in0=raw[:], in1=m1[:],
                                            op=Alu.mult)
                    nc.vector.tensor_scalar_add(out=scl[:], in0=scl[:], scalar1=1.0)
                    nc.vector.tensor_tensor(out=scl[:], in0=scl[:], in1=b_t[:],
                                            op=Alu.mult)
                    ys = spool.tile([128, TPC * C], f32, tag="xst")
                    yv = ys[:].rearrange("p (t c) -> p t c", c=C)
                    sclb = scl[:].unsqueeze(-1).broadcast_to([128, TPC, C])
                    nc.vector.tensor_tensor(out=yv, in0=xv, in1=sclb, op=Alu.mult)
                    nc.vector.tensor_add(out=ys[:], in0=ys[:], in1=a_t[:])
                    xn = ys

                if step < NSTEPS - 1:
                    xn16 = smp.tile([128, TPC * C], f16, tag="x16")
                    nc.vector.tensor_copy(out=xn16[:], in_=xn[:])
                    agin = dpool.tile([NPC, C], f16, tag="agin")
                    agv = agin[:].rearrange("(t p) c -> p t c", p=128)
                    nc.sync.dma_start(
                        out=agv, in_=xn16[:].rearrange("p (t c) -> p t c", c=C))
                    xrep = dpool.tile([M * NPC, C], f16, tag="xrep",
                                      addr_space="Shared")
                    nc.gpsimd.collective_compute(
                        "AllGather", mybir.AluOpType.bypass,
                        replica_groups=[list(range(M))],
                        ins=[agin.opt()], outs=[xrep.opt()])
                    src = xrep
                    xtab = tabp.tile([128, NWIN * C], f16, tag="xtab")
                    tv = xrep[:].rearrange("(t p) c -> p t c", p=128)
                    xtv = xtab[:].rearrange("p (t c) -> p t c", c=C)
                    for h in range(8):
                        nc.sync.dma_start(
                            out=xtv[:, h * 98:(h + 1) * 98],
                            in_=tv[:, h * 98:(h + 1) * 98])
                xcur = xn

            nc.sync.dma_start(out=out_d[:], in_=xcur[:])
    nc.compile()
    return nc


def _make_in_maps(pp):
    in_maps = []
    for k in range(M):
        pc = pp["per_core"][k]
        in_maps.append({
            "w8": pc["w8"], "nrm": pc["nrm"], "idx": pc["idx"],
            "soff": pc["soff"], "snrm": pc["snrm"], "sidx": pc["sidx"],
            "xtab0": pp["xtab0"], "xrep0": pp["xrep0"],
            "xo0": pc["xo0"], "amat": pc["amat"], "bvec": pc["bvec"],
        })
    return in_maps


TRACE = False
LAST_EXEC_NS = None
LAST_RESULTS = None


def kernel(y_true, y_soft, spread_mask, edge_index, edge_weight):
    global LAST_EXEC_NS, LAST_RESULTS
    from concourse import bass_utils

    pp = _preprocess(y_true, y_soft, spread_mask, edge_index, edge_weight)
    key = round(pp["sigma"], 9)
    if key not in _cache:
        _cache[key] = _build_program(pp["sigma"])
    nc = _cache[key]

    res = bass_utils.run_bass_kernel_spmd(nc, _make_in_maps(pp), list(range(M)),
                                          trace=TRACE)
    LAST_EXEC_NS = res.exec_time_ns
    LAST_RESULTS = res
    full = np.concatenate(
        [res.results[k]["out"].reshape(128, TPC, C).transpose(1, 0, 2)
         .reshape(NPC, C) for k in range(M)], axis=0)
    return full[pp["new_id"]].astype(np.float32)


# revision 10
# speedup vs baseline: 1.5738x; 1.5738x over previous
"""Correct-and-Smooth label propagation on 8 Trainium2 NeuronCores.

Strategy: destination-node row sharding. Nodes are relabeled (degree-balanced
round-robin across cores, degree-sorted within a core, lane-major within each
128-row tile) so each core owns a contiguous block of the replicated [N,C]
state table. Per propagation step each core:
  1. gathers source rows for its incoming edges with one indirect DMA per
     uniform-K group of destination tiles (edge slot tables are padded so the
     segment-sum becomes a fixed-stride reduction),
  2. multiplies by edge norms and segment-reduces on the vector engine,
  3. applies the alpha-mix + clip/softmax post-step,
  4. AllGathers the updated shards into the next replicated table.
"""
import numpy as np

N, E, C, M = 100000, 1600000, 40, 8
NPC, TPC = 12544, 98          # padded nodes per core, 128-row tiles per core
ALPHA_C, NUM_C = 0.9, 10
ALPHA_S, NUM_S = 0.8, 10
SLOT_CAP = 160                # max edge slots per partition per gather group
GROUP_PEN = 12
CS = 44                       # padded slot stride (elems); slot pad breaks DMA
                              # coalescing so each slot gets its own descriptor

_cache = {}


def _group_tiles(Ktile, cap=SLOT_CAP, pen=GROUP_PEN):
    T = len(Ktile)
    INF = 1 << 60
    best = [INF] * (T + 1)
    prev = [-1] * (T + 1)
    best[0] = 0
    for i in range(1, T + 1):
        mk = 0
        for j in range(i - 1, -1, -1):
            mk = max(mk, Ktile[j])
            G = i - j
            if G * mk > cap:
                break
            cst = best[j] + G * mk + pen
            if cst < best[i]:
                best[i] = cst
                prev[i] = j
    out = []
    i = T
    while i > 0:
        j = prev[i]
        out.append((j, i - j, int(max(Ktile[j:i]))))
        i = j
    return out[::-1]


def _preprocess(y_true, y_soft, spread_mask, edge_index, edge_weight):
    y_true = np.asarray(y_true)
    y_soft = np.asarray(y_soft, dtype=np.float32)
    spread_mask = np.asarray(spread_mask).astype(bool)
    row = np.asarray(edge_index[0], dtype=np.int64)
    col = np.asarray(edge_index[1], dtype=np.int64)
    w = np.asarray(edge_weight, dtype=np.float32)

    deg = np.bincount(row, weights=w.astype(np.float64), minlength=N).astype(np.float32)
    dis = np.where(deg > 0, 1.0 / np.sqrt(deg, where=deg > 0), 0.0).astype(np.float32)
    norm = (dis[row] * w * dis[col]).astype(np.float32)

    indeg = np.bincount(row, minlength=N)
    order = np.argsort(indeg, kind="stable")
    ranks = np.arange(N)
    core_of = np.empty(N, np.int64)
    pos_of = np.empty(N, np.int64)
    core_of[order] = ranks % M
    pos_of[order] = ranks // M
    lane_of = pos_of % 128
    t_of = pos_of // 128
    new_id = core_of * NPC + lane_of * TPC + t_of   # lane-major within core

    sc = new_id[col].astype(np.int32)
    destkey = (core_of[row] * TPC + t_of[row]) * 128 + lane_of[row]
    eo = np.argsort(destkey, kind="stable")
    dk_s = destkey[eo]
    sc_s = sc[eo]
    nm_s = norm[eo]
    cnt = np.bincount(dk_s, minlength=M * TPC * 128)
    starts = np.zeros(M * TPC * 128 + 1, np.int64)
    np.cumsum(cnt, out=starts[1:])
    slot = np.arange(E) - starts[dk_s]

    Ktile = cnt.reshape(M, TPC, 128).max(axis=2).max(axis=0)
    groups = _group_tiles(Ktile)
    tile_off = np.zeros(TPC, np.int64)
    off = 0
    for (t0, G, Kg) in groups:
        for t in range(t0, t0 + G):
            tile_off[t] = off + (t - t0) * Kg
        off += G * Kg
    TOT = off

    idx_all = np.zeros((M, 128, TOT), np.int32)
    nrm_all = np.zeros((M, 128, TOT), np.float32)
    e_core = dk_s // (TPC * 128)
    e_t = (dk_s // 128) % TPC
    e_lane = dk_s % 128
    epos = tile_off[e_t] + slot
    idx_all[e_core, e_lane, epos] = sc_s
    nrm_all[e_core, e_lane, epos] = nm_s

    y_oh = np.zeros((N, C), np.float32)
    y_oh[np.arange(N), y_true] = 1.0
    maskf = spread_mask[:, None]
    err = np.where(maskf, y_oh - y_soft, 0.0).astype(np.float32)
    sigma = float(np.abs(err).sum(dtype=np.float64) / spread_mask.sum())

    x0 = np.zeros((M * NPC, C), np.float32)
    x0[new_id] = err
    A = np.zeros((M * NPC, C), np.float32)
    A[new_id] = np.where(maskf, y_oh, y_soft)
    B = np.zeros((M * NPC,), np.float32)
    B[new_id] = (~spread_mask).astype(np.float32)

    return dict(idx_all=idx_all, nrm_all=nrm_all, groups=groups, TOT=TOT,
                x0=x0, A=A, B=B, sigma=sigma, new_id=new_id)


def _build_program(groups, TOT, sigma):
    import concourse.bass as bass
    import concourse.bacc as bacc
    import concourse.mybir as mybir
    import concourse.tile as tile

    f32 = mybir.dt.float32
    i32 = mybir.dt.int32
    Alu = mybir.AluOpType
    Act = mybir.ActivationFunctionType
    X = mybir.AxisListType.X

    nc = bacc.Bacc("TRN2", target_bir_lowering=False, debug=False,
                   enable_asserts=True, num_devices=M)
    x0_d = nc.dram_tensor("x0", [M * NPC, C], f32, kind="ExternalInput")
    xo0_d = nc.dram_tensor("xo0", [128, TPC * C], f32, kind="ExternalInput")
    idx_d = nc.dram_tensor("idx", [128, TOT], i32, kind="ExternalInput")
    nrm_d = nc.dram_tensor("nrm", [128, TOT], f32, kind="ExternalInput")
    a_d = nc.dram_tensor("amat", [128, TPC * C], f32, kind="ExternalInput")
    b_d = nc.dram_tensor("bvec", [128, TPC], f32, kind="ExternalInput")
    out_d = nc.dram_tensor("out", [128, TPC * C], f32, kind="ExternalOutput")

    NSTEPS = NUM_C + NUM_S
    with tile.TileContext(nc) as tc:
        with (
            tc.tile_pool(name="stat", bufs=1) as stat,
            tc.tile_pool(name="gpool", bufs=2) as gpool,
            tc.tile_pool(name="spool", bufs=3) as spool,
            tc.tile_pool(name="apool", bufs=2) as apool,
            tc.tile_pool(name="smp", bufs=2) as smp,
            tc.tile_pool(name="dpool", bufs=2, space="DRAM") as dpool,
        ):
            idx_t = stat.tile([128, TOT], i32)
            nc.sync.dma_start(out=idx_t[:], in_=idx_d[:])
            nrm_t = stat.tile([128, TOT], f32)
            nc.sync.dma_start(out=nrm_t[:], in_=nrm_d[:])
            a_t = stat.tile([128, TPC * C], f32)
            nc.sync.dma_start(out=a_t[:], in_=a_d[:])
            b_t = stat.tile([128, TPC], f32)
            nc.sync.dma_start(out=b_t[:], in_=b_d[:])
            xcur = spool.tile([128, TPC * C], f32, tag="xst")
            nc.sync.dma_start(out=xcur[:], in_=xo0_d[:])
            src = x0_d.ap()

            for step in range(NSTEPS):
                phase1 = step < NUM_C
                alpha = ALPHA_C if phase1 else ALPHA_S

                agg_t = apool.tile([128, TPC * C], f32, tag="agg")
                off = 0
                for (t0, G, Kg) in groups:
                    S = G * Kg
                    g_t = gpool.tile([128, S * C], f32, tag="gath")
                    for r in range(S):
                        nc.gpsimd.indirect_dma_start(
                            out=g_t[:, r * C:(r + 1) * C], out_offset=None, in_=src,
                            in_offset=bass.IndirectOffsetOnAxis(
                                ap=idx_t[:, off + r:off + r + 1], axis=0))
                    gv = g_t[:].rearrange("p (s c) -> p s c", c=C)
                    nv = nrm_t[:, off:off + S].unsqueeze(-1).broadcast_to([128, S, C])
                    nc.vector.tensor_tensor(out=gv, in0=gv, in1=nv, op=Alu.mult)
                    gr = g_t[:].rearrange("p (g k c) -> p g c k", k=Kg, c=C)
                    nc.vector.tensor_reduce(
                        out=agg_t[:, t0 * C:(t0 + G) * C], in_=gr, axis=X, op=Alu.add)
                    off += S

                xn = spool.tile([128, TPC * C], f32, tag="xst")
                nc.scalar.activation(xn[:], xcur[:], Act.Copy, scale=float(1 - alpha))
                nc.vector.tensor_scalar_mul(out=agg_t[:], in0=agg_t[:],
                                            scalar1=float(alpha))
                nc.vector.tensor_add(out=xn[:], in0=xn[:], in1=agg_t[:])

                if phase1:
                    nc.vector.tensor_scalar_max(out=xn[:], in0=xn[:], scalar1=-1.0)
                    nc.vector.tensor_scalar_min(out=xn[:], in0=xn[:], scalar1=1.0)
                else:
                    xv = xn[:].rearrange("p (t c) -> p t c", c=C)
                    rm = smp.tile([128, TPC], f32, tag="rm")
                    nc.vector.tensor_reduce(out=rm[:], in_=xv, axis=X, op=Alu.max)
                    rmb = rm[:].unsqueeze(-1).broadcast_to([128, TPC, C])
                    nc.vector.tensor_tensor(out=xv, in0=xv, in1=rmb, op=Alu.subtract)
                    nc.scalar.activation(xn[:], xn[:], Act.Exp)
                    ss = smp.tile([128, TPC], f32, tag="ss")
                    nc.vector.tensor_reduce(out=ss[:], in_=xv, axis=X, op=Alu.add)
                    nc.vector.reciprocal(out=ss[:], in_=ss[:])
                    ssb = ss[:].unsqueeze(-1).broadcast_to([128, TPC, C])
                    nc.vector.tensor_tensor(out=xv, in0=xv, in1=ssb, op=Alu.mult)

                if step == NUM_C - 1:
                    # correct/smooth transition: xn holds `smoothed`
                    xv = xn[:].rearrange("p (t c) -> p t c", c=C)
                    den = smp.tile([128, TPC], f32, tag="den")
                    nc.vector.tensor_reduce(out=den[:], in_=xv, axis=X, op=Alu.add,
                                            apply_absolute_value=True)
                    raw = smp.tile([128, TPC], f32, tag="raw")
                    nc.vector.reciprocal(out=raw[:], in_=den[:])
                    nc.vector.tensor_scalar_mul(out=raw[:], in0=raw[:],
                                                scalar1=float(sigma))
                    # scale = where((den>0) & (raw<=1000), raw, 1), branch-free:
                    # clamp raw first so inf (den==0) never meets a 0 multiply
                    nc.vector.tensor_scalar_min(out=raw[:], in0=raw[:],
                                                scalar1=1001.0)
                    m1 = smp.tile([128, TPC], f32, tag="m1")
                    nc.vector.tensor_scalar(out=m1[:], in0=den[:], scalar1=0.0,
                                            scalar2=None, op0=Alu.is_gt)
                    m2 = smp.tile([128, TPC], f32, tag="m2")
                    nc.vector.tensor_scalar(out=m2[:], in0=raw[:], scalar1=1000.0,
                                            scalar2=None, op0=Alu.is_le)
                    nc.vector.tensor_tensor(out=m1[:], in0=m1[:], in1=m2[:],
                                            op=Alu.mult)
                    scl = smp.tile([128, TPC], f32, tag="scl")
                    nc.vector.tensor_scalar_add(out=raw[:], in0=raw[:], scalar1=-1.0)
                    nc.vector.tensor_tensor(out=scl[:], in0=raw[:], in1=m1[:],
                                            op=Alu.mult)
                    nc.vector.tensor_scalar_add(out=scl[:], in0=scl[:], scalar1=1.0)
                    nc.vector.tensor_tensor(out=scl[:], in0=scl[:], in1=b_t[:],
                                            op=Alu.mult)
                    ys = spool.tile([128, TPC * C], f32, tag="xst")
                    yv = ys[:].rearrange("p (t c) -> p t c", c=C)
                    sclb = scl[:].unsqueeze(-1).broadcast_to([128, TPC, C])
                    nc.vector.tensor_tensor(out=yv, in0=xv, in1=sclb, op=Alu.mult)
                    nc.vector.tensor_add(out=ys[:], in0=ys[:], in1=a_t[:])
                    xn = ys

                if step < NSTEPS - 1:
                    agin = dpool.tile([128, TPC * C], f32, tag="agin")
                    nc.sync.dma_start(out=agin[:], in_=xn[:])
                    xrep = dpool.tile([M * NPC, C], f32, tag="xrep",
                                      addr_space="Shared")
                    nc.gpsimd.collective_compute(
                        "AllGather", Alu.bypass,
                        replica_groups=[list(range(M))],
                        ins=[agin.opt()], outs=[xrep.opt()])
                    src = xrep
                xcur = xn

            nc.sync.dma_start(out=out_d[:], in_=xcur[:])
    nc.compile()
    return nc


def _make_in_maps(pp):
    in_maps = []
    for k in range(M):
        xo = pp["x0"][k * NPC:(k + 1) * NPC].reshape(128, TPC * C)
        am = pp["A"][k * NPC:(k + 1) * NPC].reshape(128, TPC * C)
        bv = pp["B"][k * NPC:(k + 1) * NPC].reshape(128, TPC)
        in_maps.append({
            "x0": pp["x0"],
            "xo0": np.ascontiguousarray(xo),
            "idx": pp["idx_all"][k],
            "nrm": pp["nrm_all"][k],
            "amat": np.ascontiguousarray(am),
            "bvec": np.ascontiguousarray(bv),
        })
    return in_maps


TRACE = False
LAST_EXEC_NS = None
LAST_RESULTS = None


def kernel(y_true, y_soft, spread_mask, edge_index, edge_weight):
    global LAST_EXEC_NS, LAST_RESULTS
    from concourse import bass_utils

    pp = _preprocess(y_true, y_soft, spread_mask, edge_index, edge_weight)
    key = (tuple(pp["groups"]), pp["TOT"], round(pp["sigma"], 9))
    if key not in _cache:
        _cache[key] = _build_program(pp["groups"], pp["TOT"], pp["sigma"])
    nc = _cache[key]

    res = bass_utils.run_bass_kernel_spmd(nc, _make_in_maps(pp), list(range(M)),
                                          trace=TRACE)
    LAST_EXEC_NS = res.exec_time_ns
    LAST_RESULTS = res
    full = np.concatenate(
        [res.results[k]["out"].reshape(NPC, C) for k in range(M)], axis=0)
    return full[pp["new_id"]].astype(np.float32)



# revision 11
# speedup vs baseline: 1.5988x; 1.0159x over previous
"""Correct-and-Smooth label propagation on 8 Trainium2 NeuronCores.

Strategy: destination-node row sharding. Nodes are relabeled (degree-balanced
round-robin across cores, degree-sorted within a core, lane-major within each
128-row tile) so each core owns a contiguous block of the replicated [N,C]
state table. Per propagation step each core:
  1. gathers source rows for its incoming edges with one indirect DMA per
     uniform-K group of destination tiles (edge slot tables are padded so the
     segment-sum becomes a fixed-stride reduction),
  2. multiplies by edge norms and segment-reduces on the vector engine,
  3. applies the alpha-mix + clip/softmax post-step,
  4. AllGathers the updated shards into the next replicated table.
"""
import numpy as np

N, E, C, M = 100000, 1600000, 40, 8
NPC, TPC = 12544, 98          # padded nodes per core, 128-row tiles per core
ALPHA_C, NUM_C = 0.9, 10
ALPHA_S, NUM_S = 0.8, 10
SLOT_CAP = 160                # max edge slots per partition per gather group
GROUP_PEN = 12
CS = 44                       # padded slot stride (elems); slot pad breaks DMA
                              # coalescing so each slot gets its own descriptor

_cache = {}


def _group_tiles(Ktile, cap=SLOT_CAP, pen=GROUP_PEN):
    T = len(Ktile)
    INF = 1 << 60
    best = [INF] * (T + 1)
    prev = [-1] * (T + 1)
    best[0] = 0
    for i in range(1, T + 1):
        mk = 0
        for j in range(i - 1, -1, -1):
            mk = max(mk, Ktile[j])
            G = i - j
            if G * mk > cap:
                break
            cst = best[j] + G * mk + pen
            if cst < best[i]:
                best[i] = cst
                prev[i] = j
    out = []
    i = T
    while i > 0:
        j = prev[i]
        out.append((j, i - j, int(max(Ktile[j:i]))))
        i = j
    return out[::-1]


def _preprocess(y_true, y_soft, spread_mask, edge_index, edge_weight):
    y_true = np.asarray(y_true)
    y_soft = np.asarray(y_soft, dtype=np.float32)
    spread_mask = np.asarray(spread_mask).astype(bool)
    row = np.asarray(edge_index[0], dtype=np.int64)
    col = np.asarray(edge_index[1], dtype=np.int64)
    w = np.asarray(edge_weight, dtype=np.float32)

    deg = np.bincount(row, weights=w.astype(np.float64), minlength=N).astype(np.float32)
    dis = np.where(deg > 0, 1.0 / np.sqrt(deg, where=deg > 0), 0.0).astype(np.float32)
    norm = (dis[row] * w * dis[col]).astype(np.float32)

    indeg = np.bincount(row, minlength=N)
    order = np.argsort(indeg, kind="stable")
    ranks = np.arange(N)
    core_of = np.empty(N, np.int64)
    pos_of = np.empty(N, np.int64)
    core_of[order] = ranks % M
    pos_of[order] = ranks // M
    lane_of = pos_of % 128
    t_of = pos_of // 128
    new_id = core_of * NPC + lane_of * TPC + t_of   # lane-major within core

    sc = new_id[col].astype(np.int32)
    destkey = (core_of[row] * TPC + t_of[row]) * 128 + lane_of[row]
    eo = np.argsort(destkey, kind="stable")
    dk_s = destkey[eo]
    sc_s = sc[eo]
    nm_s = norm[eo]
    cnt = np.bincount(dk_s, minlength=M * TPC * 128)
    starts = np.zeros(M * TPC * 128 + 1, np.int64)
    np.cumsum(cnt, out=starts[1:])
    slot = np.arange(E) - starts[dk_s]

    Ktile = cnt.reshape(M, TPC, 128).max(axis=2).max(axis=0)
    groups = _group_tiles(Ktile)
    tile_off = np.zeros(TPC, np.int64)
    off = 0
    for (t0, G, Kg) in groups:
        for t in range(t0, t0 + G):
            tile_off[t] = off + (t - t0) * Kg
        off += G * Kg
    TOT = off

    idx_all = np.zeros((M, 128, TOT), np.int32)
    nrm_all = np.zeros((M, 128, TOT), np.float32)
    e_core = dk_s // (TPC * 128)
    e_t = (dk_s // 128) % TPC
    e_lane = dk_s % 128
    epos = tile_off[e_t] + slot
    idx_all[e_core, e_lane, epos] = sc_s
    nrm_all[e_core, e_lane, epos] = nm_s

    y_oh = np.zeros((N, C), np.float32)
    y_oh[np.arange(N), y_true] = 1.0
    maskf = spread_mask[:, None]
    err = np.where(maskf, y_oh - y_soft, 0.0).astype(np.float32)
    sigma = float(np.abs(err).sum(dtype=np.float64) / spread_mask.sum())

    x0 = np.zeros((M * NPC, C), np.float32)
    x0[new_id] = err
    A = np.zeros((M * NPC, C), np.float32)
    A[new_id] = np.where(maskf, y_oh, y_soft)
    B = np.zeros((M * NPC,), np.float32)
    B[new_id] = (~spread_mask).astype(np.float32)

    return dict(idx_all=idx_all, nrm_all=nrm_all, groups=groups, TOT=TOT,
                x0=x0, A=A, B=B, sigma=sigma, new_id=new_id)


def _build_program(groups, TOT, sigma):
    import concourse.bass as bass
    import concourse.bacc as bacc
    import concourse.mybir as mybir
    import concourse.tile as tile

    f32 = mybir.dt.float32
    f16 = mybir.dt.float16
    i32 = mybir.dt.int32
    Alu = mybir.AluOpType
    Act = mybir.ActivationFunctionType
    X = mybir.AxisListType.X

    nc = bacc.Bacc("TRN2", target_bir_lowering=False, debug=False,
                   enable_asserts=True, num_devices=M)
    x0_d = nc.dram_tensor("x0", [M * NPC, C], f16, kind="ExternalInput")
    xo0_d = nc.dram_tensor("xo0", [128, TPC * C], f32, kind="ExternalInput")
    idx_d = nc.dram_tensor("idx", [128, TOT], i32, kind="ExternalInput")
    nrm_d = nc.dram_tensor("nrm", [128, TOT], f16, kind="ExternalInput")
    a_d = nc.dram_tensor("amat", [128, TPC * C], f32, kind="ExternalInput")
    b_d = nc.dram_tensor("bvec", [128, TPC], f32, kind="ExternalInput")
    out_d = nc.dram_tensor("out", [128, TPC * C], f32, kind="ExternalOutput")

    NSTEPS = NUM_C + NUM_S
    with tile.TileContext(nc) as tc:
        with (
            tc.tile_pool(name="stat", bufs=1) as stat,
            tc.tile_pool(name="gpool", bufs=2) as gpool,
            tc.tile_pool(name="spool", bufs=3) as spool,
            tc.tile_pool(name="apool", bufs=2) as apool,
            tc.tile_pool(name="smp", bufs=2) as smp,
            tc.tile_pool(name="dpool", bufs=2, space="DRAM") as dpool,
        ):
            idx_t = stat.tile([128, TOT], i32)
            nc.sync.dma_start(out=idx_t[:], in_=idx_d[:])
            nrm_t = stat.tile([128, TOT], f16)
            nc.sync.dma_start(out=nrm_t[:], in_=nrm_d[:])
            a_t = stat.tile([128, TPC * C], f32)
            nc.sync.dma_start(out=a_t[:], in_=a_d[:])
            b_t = stat.tile([128, TPC], f32)
            nc.sync.dma_start(out=b_t[:], in_=b_d[:])
            xcur = spool.tile([128, TPC * C], f32, tag="xst")
            nc.sync.dma_start(out=xcur[:], in_=xo0_d[:])
            src = x0_d.ap()

            for step in range(NSTEPS):
                phase1 = step < NUM_C
                alpha = ALPHA_C if phase1 else ALPHA_S

                agg_t = apool.tile([128, TPC * C], f32, tag="agg")
                off = 0
                for (t0, G, Kg) in groups:
                    S = G * Kg
                    g_t = gpool.tile([128, S * C], f16, tag="gath")
                    for r in range(S):
                        nc.gpsimd.indirect_dma_start(
                            out=g_t[:, r * C:(r + 1) * C], out_offset=None, in_=src,
                            in_offset=bass.IndirectOffsetOnAxis(
                                ap=idx_t[:, off + r:off + r + 1], axis=0))
                    gv = g_t[:].rearrange("p (s c) -> p s c", c=C)
                    nv = nrm_t[:, off:off + S].unsqueeze(-1).broadcast_to([128, S, C])
                    nc.vector.tensor_tensor(out=gv, in0=gv, in1=nv, op=Alu.mult)
                    gr = g_t[:].rearrange("p (g k c) -> p g c k", k=Kg, c=C)
                    nc.vector.tensor_reduce(
                        out=agg_t[:, t0 * C:(t0 + G) * C], in_=gr, axis=X, op=Alu.add)
                    off += S

                xn = spool.tile([128, TPC * C], f32, tag="xst")
                nc.scalar.activation(xn[:], xcur[:], Act.Copy, scale=float(1 - alpha))
                nc.vector.tensor_scalar_mul(out=agg_t[:], in0=agg_t[:],
                                            scalar1=float(alpha))
                nc.vector.tensor_add(out=xn[:], in0=xn[:], in1=agg_t[:])

                if phase1:
                    nc.vector.tensor_scalar_max(out=xn[:], in0=xn[:], scalar1=-1.0)
                    nc.vector.tensor_scalar_min(out=xn[:], in0=xn[:], scalar1=1.0)
                else:
                    xv = xn[:].rearrange("p (t c) -> p t c", c=C)
                    nc.scalar.activation(xn[:], xn[:], Act.Exp)
                    ss = smp.tile([128, TPC], f32, tag="ss")
                    nc.vector.tensor_reduce(out=ss[:], in_=xv, axis=X, op=Alu.add)
                    nc.vector.reciprocal(out=ss[:], in_=ss[:])
                    ssb = ss[:].unsqueeze(-1).broadcast_to([128, TPC, C])
                    nc.vector.tensor_tensor(out=xv, in0=xv, in1=ssb, op=Alu.mult)

                if step == NUM_C - 1:
                    # correct/smooth transition: xn holds `smoothed`
                    xv = xn[:].rearrange("p (t c) -> p t c", c=C)
                    den = smp.tile([128, TPC], f32, tag="den")
                    nc.vector.tensor_reduce(out=den[:], in_=xv, axis=X, op=Alu.add,
                                            apply_absolute_value=True)
                    raw = smp.tile([128, TPC], f32, tag="raw")
                    nc.vector.reciprocal(out=raw[:], in_=den[:])
                    nc.vector.tensor_scalar_mul(out=raw[:], in0=raw[:],
                                                scalar1=float(sigma))
                    # scale = where((den>0) & (raw<=1000), raw, 1), branch-free:
                    # clamp raw first so inf (den==0) never meets a 0 multiply
                    nc.vector.tensor_scalar_min(out=raw[:], in0=raw[:],
                                                scalar1=1001.0)
                    m1 = smp.tile([128, TPC], f32, tag="m1")
                    nc.vector.tensor_scalar(out=m1[:], in0=den[:], scalar1=0.0,
                                            scalar2=None, op0=Alu.is_gt)
                    m2 = smp.tile([128, TPC], f32, tag="m2")
                    nc.vector.tensor_scalar(out=m2[:], in0=raw[:], scalar1=1000.0,
                                            scalar2=None, op0=Alu.is_le)
                    nc.vector.tensor_tensor(out=m1[:], in0=m1[:], in1=m2[:],
                                            op=Alu.mult)
                    scl = smp.tile([128, TPC], f32, tag="scl")
                    nc.vector.tensor_scalar_add(out=raw[:], in0=raw[:], scalar1=-1.0)
                    nc.vector.tensor_tensor(out=scl[:], in0=raw[:], in1=m1[:],
                                            op=Alu.mult)
                    nc.vector.tensor_scalar_add(out=scl[:], in0=scl[:], scalar1=1.0)
                    nc.vector.tensor_tensor(out=scl[:], in0=scl[:], in1=b_t[:],
                                            op=Alu.mult)
                    ys = spool.tile([128, TPC * C], f32, tag="xst")
                    yv = ys[:].rearrange("p (t c) -> p t c", c=C)
                    sclb = scl[:].unsqueeze(-1).broadcast_to([128, TPC, C])
                    nc.vector.tensor_tensor(out=yv, in0=xv, in1=sclb, op=Alu.mult)
                    nc.vector.tensor_add(out=ys[:], in0=ys[:], in1=a_t[:])
                    xn = ys

                if step < NSTEPS - 1:
                    xn16 = smp.tile([128, TPC * C], f16, tag="x16")
                    nc.vector.tensor_copy(out=xn16[:], in_=xn[:])
                    agin = dpool.tile([128, TPC * C], f16, tag="agin")
                    nc.sync.dma_start(out=agin[:], in_=xn16[:])
                    xrep = dpool.tile([M * NPC, C], f16, tag="xrep",
                                      addr_space="Shared")
                    nc.gpsimd.collective_compute(
                        "AllGather", Alu.bypass,
                        replica_groups=[list(range(M))],
                        ins=[agin.opt()], outs=[xrep.opt()])
                    src = xrep
                xcur = xn

            nc.sync.dma_start(out=out_d[:], in_=xcur[:])
    nc.compile()
    return nc


def _make_in_maps(pp):
    in_maps = []
    for k in range(M):
        xo = pp["x0"][k * NPC:(k + 1) * NPC].reshape(128, TPC * C)
        am = pp["A"][k * NPC:(k + 1) * NPC].reshape(128, TPC * C)
        bv = pp["B"][k * NPC:(k + 1) * NPC].reshape(128, TPC)
        in_maps.append({
            "x0": pp["x0"].astype(np.float16),
            "xo0": np.ascontiguousarray(xo),
            "idx": pp["idx_all"][k],
            "nrm": pp["nrm_all"][k].astype(np.float16),
            "amat": np.ascontiguousarray(am),
            "bvec": np.ascontiguousarray(bv),
        })
    return in_maps


TRACE = False
LAST_EXEC_NS = None
LAST_RESULTS = None


def kernel(y_true, y_soft, spread_mask, edge_index, edge_weight):
    global LAST_EXEC_NS, LAST_RESULTS
    from concourse import bass_utils

    pp = _preprocess(y_true, y_soft, spread_mask, edge_index, edge_weight)
    key = (tuple(pp["groups"]), pp["TOT"], round(pp["sigma"], 9))
    if key not in _cache:
        _cache[key] = _build_program(pp["groups"], pp["TOT"], pp["sigma"])
    nc = _cache[key]

    res = bass_utils.run_bass_kernel_spmd(nc, _make_in_maps(pp), list(range(M)),
                                          trace=TRACE)
    LAST_EXEC_NS = res.exec_time_ns
    LAST_RESULTS = res
    full = np.concatenate(
        [res.results[k]["out"].reshape(NPC, C) for k in range(M)], axis=0)
    return full[pp["new_id"]].astype(np.float32)



# revision 12
# speedup vs baseline: 1.6190x; 1.0126x over previous
"""Correct-and-Smooth label propagation on 8 Trainium2 NeuronCores.

Strategy: destination-node row sharding. Nodes are relabeled (degree-balanced
round-robin across cores, degree-sorted within a core, lane-major within each
128-row tile) so each core owns a contiguous block of the replicated [N,C]
state table. Per propagation step each core:
  1. gathers source rows for its incoming edges with one indirect DMA per
     uniform-K group of destination tiles (edge slot tables are padded so the
     segment-sum becomes a fixed-stride reduction),
  2. multiplies by edge norms and segment-reduces on the vector engine,
  3. applies the alpha-mix + clip/softmax post-step,
  4. AllGathers the updated shards into the next replicated table.
"""
import numpy as np

N, E, C, M = 100000, 1600000, 40, 8
NPC, TPC = 12544, 98          # padded nodes per core, 128-row tiles per core
ALPHA_C, NUM_C = 0.9, 10
ALPHA_S, NUM_S = 0.8, 10
SLOT_CAP = 160                # max edge slots per partition per gather group
GROUP_PEN = 2
CS = 44                       # padded slot stride (elems); slot pad breaks DMA
                              # coalescing so each slot gets its own descriptor

_cache = {}


def _group_tiles(Ktile, cap=SLOT_CAP, pen=GROUP_PEN):
    T = len(Ktile)
    INF = 1 << 60
    best = [INF] * (T + 1)
    prev = [-1] * (T + 1)
    best[0] = 0
    for i in range(1, T + 1):
        mk = 0
        for j in range(i - 1, -1, -1):
            mk = max(mk, Ktile[j])
            G = i - j
            if G * mk > cap:
                break
            cst = best[j] + G * mk + pen
            if cst < best[i]:
                best[i] = cst
                prev[i] = j
    out = []
    i = T
    while i > 0:
        j = prev[i]
        out.append((j, i - j, int(max(Ktile[j:i]))))
        i = j
    return out[::-1]


def _preprocess(y_true, y_soft, spread_mask, edge_index, edge_weight):
    y_true = np.asarray(y_true)
    y_soft = np.asarray(y_soft, dtype=np.float32)
    spread_mask = np.asarray(spread_mask).astype(bool)
    row = np.asarray(edge_index[0], dtype=np.int64)
    col = np.asarray(edge_index[1], dtype=np.int64)
    w = np.asarray(edge_weight, dtype=np.float32)

    deg = np.bincount(row, weights=w.astype(np.float64), minlength=N).astype(np.float32)
    dis = np.where(deg > 0, 1.0 / np.sqrt(deg, where=deg > 0), 0.0).astype(np.float32)
    norm = (dis[row] * w * dis[col]).astype(np.float32)

    indeg = np.bincount(row, minlength=N)
    order = np.argsort(indeg, kind="stable")
    ranks = np.arange(N)
    core_of = np.empty(N, np.int64)
    pos_of = np.empty(N, np.int64)
    core_of[order] = ranks % M
    pos_of[order] = ranks // M
    lane_of = pos_of % 128
    t_of = pos_of // 128
    new_id = core_of * NPC + lane_of * TPC + t_of   # lane-major within core

    sc = new_id[col].astype(np.int32)
    destkey = (core_of[row] * TPC + t_of[row]) * 128 + lane_of[row]
    eo = np.argsort(destkey, kind="stable")
    dk_s = destkey[eo]
    sc_s = sc[eo]
    nm_s = norm[eo]
    cnt = np.bincount(dk_s, minlength=M * TPC * 128)
    starts = np.zeros(M * TPC * 128 + 1, np.int64)
    np.cumsum(cnt, out=starts[1:])
    slot = np.arange(E) - starts[dk_s]

    Ktile = cnt.reshape(M, TPC, 128).max(axis=2).max(axis=0)
    groups = _group_tiles(Ktile)
    tile_off = np.zeros(TPC, np.int64)
    off = 0
    for (t0, G, Kg) in groups:
        for t in range(t0, t0 + G):
            tile_off[t] = off + (t - t0) * Kg
        off += G * Kg
    TOT = off

    idx_all = np.zeros((M, 128, TOT), np.int32)
    nrm_all = np.zeros((M, 128, TOT), np.float32)
    e_core = dk_s // (TPC * 128)
    e_t = (dk_s // 128) % TPC
    e_lane = dk_s % 128
    epos = tile_off[e_t] + slot
    idx_all[e_core, e_lane, epos] = sc_s
    nrm_all[e_core, e_lane, epos] = nm_s

    y_oh = np.zeros((N, C), np.float32)
    y_oh[np.arange(N), y_true] = 1.0
    maskf = spread_mask[:, None]
    err = np.where(maskf, y_oh - y_soft, 0.0).astype(np.float32)
    sigma = float(np.abs(err).sum(dtype=np.float64) / spread_mask.sum())

    x0 = np.zeros((M * NPC, C), np.float32)
    x0[new_id] = err
    A = np.zeros((M * NPC, C), np.float32)
    A[new_id] = np.where(maskf, y_oh, y_soft)
    B = np.zeros((M * NPC,), np.float32)
    B[new_id] = (~spread_mask).astype(np.float32)

    return dict(idx_all=idx_all, nrm_all=nrm_all, groups=groups, TOT=TOT,
                x0=x0, A=A, B=B, sigma=sigma, new_id=new_id)


def _build_program(groups, TOT, sigma):
    import concourse.bass as bass
    import concourse.bacc as bacc
    import concourse.mybir as mybir
    import concourse.tile as tile

    f32 = mybir.dt.float32
    f16 = mybir.dt.float16
    i32 = mybir.dt.int32
    Alu = mybir.AluOpType
    Act = mybir.ActivationFunctionType
    X = mybir.AxisListType.X

    nc = bacc.Bacc("TRN2", target_bir_lowering=False, debug=False,
                   enable_asserts=True, num_devices=M)
    x0_d = nc.dram_tensor("x0", [M * NPC, C], f16, kind="ExternalInput")
    xo0_d = nc.dram_tensor("xo0", [128, TPC * C], f32, kind="ExternalInput")
    idx_d = nc.dram_tensor("idx", [128, TOT], i32, kind="ExternalInput")
    nrm_d = nc.dram_tensor("nrm", [128, TOT], f16, kind="ExternalInput")
    a_d = nc.dram_tensor("amat", [128, TPC * C], f32, kind="ExternalInput")
    b_d = nc.dram_tensor("bvec", [128, TPC], f32, kind="ExternalInput")
    out_d = nc.dram_tensor("out", [128, TPC * C], f32, kind="ExternalOutput")

    NSTEPS = NUM_C + NUM_S
    with tile.TileContext(nc) as tc:
        with (
            tc.tile_pool(name="stat", bufs=1) as stat,
            tc.tile_pool(name="gpool", bufs=3) as gpool,
            tc.tile_pool(name="spool", bufs=3) as spool,
            tc.tile_pool(name="apool", bufs=2) as apool,
            tc.tile_pool(name="smp", bufs=2) as smp,
            tc.tile_pool(name="dpool", bufs=2, space="DRAM") as dpool,
        ):
            idx_t = stat.tile([128, TOT], i32)
            nc.sync.dma_start(out=idx_t[:], in_=idx_d[:])
            nrm_t = stat.tile([128, TOT], f16)
            nc.sync.dma_start(out=nrm_t[:], in_=nrm_d[:])
            a_t = stat.tile([128, TPC * C], f32)
            nc.sync.dma_start(out=a_t[:], in_=a_d[:])
            b_t = stat.tile([128, TPC], f32)
            nc.sync.dma_start(out=b_t[:], in_=b_d[:])
            xcur = spool.tile([128, TPC * C], f32, tag="xst")
            nc.sync.dma_start(out=xcur[:], in_=xo0_d[:])
            src = x0_d.ap()

            for step in range(NSTEPS):
                phase1 = step < NUM_C
                alpha = ALPHA_C if phase1 else ALPHA_S

                agg_t = apool.tile([128, TPC * C], f32, tag="agg")
                off = 0
                for (t0, G, Kg) in groups:
                    S = G * Kg
                    g_t = gpool.tile([128, S * C], f16, tag="gath")
                    for r in range(S):
                        nc.gpsimd.indirect_dma_start(
                            out=g_t[:, r * C:(r + 1) * C], out_offset=None, in_=src,
                            in_offset=bass.IndirectOffsetOnAxis(
                                ap=idx_t[:, off + r:off + r + 1], axis=0))
                    gv = g_t[:].rearrange("p (s c) -> p s c", c=C)
                    nv = nrm_t[:, off:off + S].unsqueeze(-1).broadcast_to([128, S, C])
                    nc.vector.tensor_tensor(out=gv, in0=gv, in1=nv, op=Alu.mult)
                    gr = g_t[:].rearrange("p (g k c) -> p g c k", k=Kg, c=C)
                    nc.vector.tensor_reduce(
                        out=agg_t[:, t0 * C:(t0 + G) * C], in_=gr, axis=X, op=Alu.add)
                    off += S

                xn = spool.tile([128, TPC * C], f32, tag="xst")
                nc.scalar.activation(xn[:], xcur[:], Act.Copy, scale=float(1 - alpha))
                nc.vector.tensor_scalar_mul(out=agg_t[:], in0=agg_t[:],
                                            scalar1=float(alpha))
                nc.vector.tensor_add(out=xn[:], in0=xn[:], in1=agg_t[:])

                if phase1:
                    nc.vector.tensor_scalar_max(out=xn[:], in0=xn[:], scalar1=-1.0)
                    nc.vector.tensor_scalar_min(out=xn[:], in0=xn[:], scalar1=1.0)
                else:
                    xv = xn[:].rearrange("p (t c) -> p t c", c=C)
                    nc.scalar.activation(xn[:], xn[:], Act.Exp)
                    ss = smp.tile([128, TPC], f32, tag="ss")
                    nc.vector.tensor_reduce(out=ss[:], in_=xv, axis=X, op=Alu.add)
                    nc.vector.reciprocal(out=ss[:], in_=ss[:])
                    ssb = ss[:].unsqueeze(-1).broadcast_to([128, TPC, C])
                    nc.vector.tensor_tensor(out=xv, in0=xv, in1=ssb, op=Alu.mult)

                if step == NUM_C - 1:
                    # correct/smooth transition: xn holds `smoothed`
                    xv = xn[:].rearrange("p (t c) -> p t c", c=C)
                    den = smp.tile([128, TPC], f32, tag="den")
                    nc.vector.tensor_reduce(out=den[:], in_=xv, axis=X, op=Alu.add,
                                            apply_absolute_value=True)
                    raw = smp.tile([128, TPC], f32, tag="raw")
                    nc.vector.reciprocal(out=raw[:], in_=den[:])
                    nc.vector.tensor_scalar_mul(out=raw[:], in0=raw[:],
                                                scalar1=float(sigma))
                    # scale = where((den>0) & (raw<=1000), raw, 1), branch-free:
                    # clamp raw first so inf (den==0) never meets a 0 multiply
                    nc.vector.tensor_scalar_min(out=raw[:], in0=raw[:],
                                                scalar1=1001.0)
                    m1 = smp.tile([128, TPC], f32, tag="m1")
                    nc.vector.tensor_scalar(out=m1[:], in0=den[:], scalar1=0.0,
                                            scalar2=None, op0=Alu.is_gt)
                    m2 = smp.tile([128, TPC], f32, tag="m2")
                    nc.vector.tensor_scalar(out=m2[:], in0=raw[:], scalar1=1000.0,
                                            scalar2=None, op0=Alu.is_le)
                    nc.vector.tensor_tensor(out=m1[:], in0=m1[:], in1=m2[:],
                                            op=Alu.mult)
                    scl = smp.tile([128, TPC], f32, tag="scl")
                    nc.vector.tensor_scalar_add(out=raw[:], in0=raw[:], scalar1=-1.0)
                    nc.vector.tensor_tensor(out=scl[:], in0=raw[:], in1=m1[:],
                                            op=Alu.mult)
                    nc.vector.tensor_scalar_add(out=scl[:], in0=scl[:], scalar1=1.0)
                    nc.vector.tensor_tensor(out=scl[:], in0=scl[:], in1=b_t[:],
                                            op=Alu.mult)
                    ys = spool.tile([128, TPC * C], f32, tag="xst")
                    yv = ys[:].rearrange("p (t c) -> p t c", c=C)
                    sclb = scl[:].unsqueeze(-1).broadcast_to([128, TPC, C])
                    nc.vector.tensor_tensor(out=yv, in0=xv, in1=sclb, op=Alu.mult)
                    nc.vector.tensor_add(out=ys[:], in0=ys[:], in1=a_t[:])
                    xn = ys

                if step < NSTEPS - 1:
                    xn16 = smp.tile([128, TPC * C], f16, tag="x16")
                    nc.vector.tensor_copy(out=xn16[:], in_=xn[:])
                    agin = dpool.tile([128, TPC * C], f16, tag="agin")
                    nc.sync.dma_start(out=agin[:], in_=xn16[:])
                    xrep = dpool.tile([M * NPC, C], f16, tag="xrep",
                                      addr_space="Shared")
                    nc.gpsimd.collective_compute(
                        "AllGather", Alu.bypass,
                        replica_groups=[list(range(M))],
                        ins=[agin.opt()], outs=[xrep.opt()])
                    src = xrep
                xcur = xn

            nc.sync.dma_start(out=out_d[:], in_=xcur[:])
    nc.compile()
    return nc


def _make_in_maps(pp):
    in_maps = []
    for k in range(M):
        xo = pp["x0"][k * NPC:(k + 1) * NPC].reshape(128, TPC * C)
        am = pp["A"][k * NPC:(k + 1) * NPC].reshape(128, TPC * C)
        bv = pp["B"][k * NPC:(k + 1) * NPC].reshape(128, TPC)
        in_maps.append({
            "x0": pp["x0"].astype(np.float16),
            "xo0": np.ascontiguousarray(xo),
            "idx": pp["idx_all"][k],
            "nrm": pp["nrm_all"][k].astype(np.float16),
            "amat": np.ascontiguousarray(am),
            "bvec": np.ascontiguousarray(bv),
        })
    return in_maps


TRACE = False
LAST_EXEC_NS = None
LAST_RESULTS = None


def kernel(y_true, y_soft, spread_mask, edge_index, edge_weight):
    global LAST_EXEC_NS, LAST_RESULTS
    from concourse import bass_utils

    pp = _preprocess(y_true, y_soft, spread_mask, edge_index, edge_weight)
    key = (tuple(pp["groups"]), pp["TOT"], round(pp["sigma"], 9))
    if key not in _cache:
        _cache[key] = _build_program(pp["groups"], pp["TOT"], pp["sigma"])
    nc = _cache[key]

    res = bass_utils.run_bass_kernel_spmd(nc, _make_in_maps(pp), list(range(M)),
                                          trace=TRACE)
    LAST_EXEC_NS = res.exec_time_ns
    LAST_RESULTS = res
    full = np.concatenate(
        [res.results[k]["out"].reshape(NPC, C) for k in range(M)], axis=0)
    return full[pp["new_id"]].astype(np.float32)

